# revision 1
# baseline (speedup 1.0000x reference)
"""BitConvSwiGLU on 8 Trainium2 cores.

Strategy: pure token-data-parallelism. The 8192 tokens (B*S) are split into
8 slabs of 1024 tokens; each core computes its slab end-to-end (both
matmuls over the full d_hidden) so no collectives are needed. The depthwise
conv needs one halo token on each side, recomputed locally from a
halo-padded x slab (zero rows at batch boundaries reproduce the conv's
zero padding, since bit_linear(0) == 0).

Numerics: act_quant/weight_quant produce integer-valued tensors
(int8 range / ternary). Integers up to 127 are exact in bf16 and their
products accumulate exactly in fp32 PSUM (sums < 2^24), so both matmuls
run at full bf16 PE rate while matching the fp32 reference bit-closely.
fp32 -> int8 tensor-op output conversion saturates and rounds to nearest
even, which is exactly clip(round(x), -128, 127).

Engine placement: matmuls + transposes on PE; dequant / conv adds /
quantize on DVE; conv taps (Copy w/ per-channel scale), Silu(+bias) and
Abs on ACT batched in groups of 8 channel-chunks so the activation table
isn't reloaded per op; int8->bf16 widening copies on the otherwise-idle
GpSimd.
"""
import math
from contextlib import ExitStack

import numpy as np
import ml_dtypes


# ---------------------------------------------------------------------------
# Workaround: this walrus build rejects >1 sync wait on CTRL-class
# instructions (Drain/Nop). TileContext's epilogue drain aggregates one wait
# per active proc onto a single Drain. Split the excess onto follow-up nops.
def _install_tile_patch():
    import concourse.mybir as mybir
    from concourse.tile import TileContext
    from concourse.vector_clock import ScopedClock

    if getattr(TileContext, "_drain_patch_installed", False):
        return

    MAX_WAITS = 1

    def _split_waits(nc, inst):
        si = inst.ins.sync_info
        if si is None or len(si.on_wait) <= MAX_WAITS:
            return
        waits = list(si.on_wait)
        si.on_wait = waits[:MAX_WAITS]
        inst.ins.sync_info = si
        for i in range(MAX_WAITS, len(waits), MAX_WAITS):
            nop = nc.sync.nop()
            nop.ins.sync_info = mybir.SyncInfo(
                on_wait=waits[i : i + MAX_WAITS], on_update=[]
            )

    def _patched_drain_and_barrier(self, tick_clock, wait_clock):
        nc = self.nc
        drain_inst = nc.sync.drain()
        wait_clock.add_sem_waits(
            drain_inst.ins, ScopedClock({None: tick_clock.global_clock})
        )
        _split_waits(nc, drain_inst)

        nc.all_engine_barrier()
        assert self.sems is not None
        popped = nc._tile_sem_poison_stack.pop()
        assert popped is self._sem_poison
        nc.clear_and_free_semaphores(list(self.sems.allocated().values()))
        nc.all_engine_barrier()

    TileContext._drain_and_barrier = _patched_drain_and_barrier
    TileContext._drain_patch_installed = True

    # Generic safety net: rewrite the BIR JSON before compile, splitting any
    # instruction with >1 sync wait into same-engine NoOps placed before it
    # (a same-engine nop stalls the engine identically, so semantics hold).
    import json as _json
    import concourse.bass_utils as _bu
    import concourse.bass2jax as _b2j

    _orig_compile = _bu.compile_bir_kernel

    def _split_bir_waits(bir_json: bytes) -> bytes:
        d = _json.loads(bir_json)
        n_split = [0]

        def fix_block(b):
            insts = b.get("instructions", [])
            out = []
            for inst in insts:
                si = inst.get("sync_info")
                waits = si.get("on_wait") if si else None
                if waits and len(waits) > 1:
                    keep, extra = waits[:1], waits[1:]
                    for j in range(0, len(extra)):
                        out.append({
                            "name": f"{inst['name']}_w{j}",
                            "opcode": "NoOp",
                            "engine": inst.get("engine", "SP"),
                            "ins": [],
                            "outs": [],
                            "sync_info": {
                                "on_wait": [extra[j]],
                                "on_update": [],
                            },
                        })
                        n_split[0] += 1
                    si["on_wait"] = keep
                out.append(inst)
            b["instructions"] = out
            for sub in b.get("blocks", []):
                fix_block(sub)

        for f in d.get("functions", []):
            for b in f.get("blocks", []):
                fix_block(b)
        if n_split[0]:
            return _json.dumps(d).encode()
        return bir_json

    def _patched_compile(bir_json, tmpdir, neff_name="file.neff"):
        return _orig_compile(_split_bir_waits(bir_json), tmpdir, neff_name)

    _bu.compile_bir_kernel = _patched_compile
    _b2j.compile_bir_kernel = _patched_compile


# ---------------------------------------------------------------------------
# Problem dims (hardcoded per contract)
B, S, D, H = 4, 2048, 1024, 4096
N_CORES = 8
EPS = 1e-5
P = 128
GSZ = 8  # channel-chunks per ACT phase group


def _split512(n):
    k = math.ceil(n / 512)
    base, rem = divmod(n, k)
    return [base + (1 if i < rem else 0) for i in range(k)]


def build_nc(t_own, alpha_c, beta_c, sim_silu=False):
    """Build the SPMD single-core program for a slab of t_own tokens."""
    import concourse.bass as bass
    import concourse.mybir as mybir
    from concourse.tile import TileContext
    from concourse.masks import make_identity
    from concourse import bass_isa
    from concourse import library_config

    f32 = mybir.dt.float32
    bf16 = mybir.dt.bfloat16
    i8 = mybir.dt.int8
    AF = mybir.ActivationFunctionType
    ALU = mybir.AluOpType
    AX = mybir.AxisListType

    assert t_own % 256 == 0
    half = t_own // 2
    hext = half + 2
    n1 = hext // 2          # mm1 moving-dim chunk (2 chunks per half)
    mt = half // P          # output M tiles per half
    text = t_own + 2
    tt = math.ceil(text / P)  # token tiles for x load/quant
    dc = D // P             # K chunks for mm1
    cc = H // P             # hidden-channel chunks

    nc = bass.Bass()
    xe = nc.declare_dram_parameter("xe", [text, D], f32, isOutput=False)
    w1s = nc.declare_dram_parameter("w1s", [cc, P, D], bf16, isOutput=False)
    w2t = nc.declare_dram_parameter("w2t", [H, D], bf16, isOutput=False)
    cw = nc.declare_dram_parameter("cw", [H, 4], f32, isOutput=False)
    y_ext = nc.declare_dram_parameter("y", [t_own, D], f32, isOutput=True)
    hspill = nc.dram_tensor("hspill", [2, H, half], f32)

    ctx = ExitStack()
    with TileContext(nc) as tc, ctx:
        pool = lambda name, bufs, space="SBUF": ctx.enter_context(
            tc.tile_pool(name=name, bufs=bufs, space=space)
        )
        const = pool("const", 1)
        res = pool("resident", 1)
        xqt_pool = pool("xqt", dc)
        xload = pool("xload", 2)
        xstat = pool("xstat", 4)
        w1p = pool("w1p", 3)
        w2p = pool("w2p", 3)
        cwp = pool("cwp", 3 * GSZ)
        hpool = pool("hload", 3)
        acttmp = pool("acttmp", 6)
        q8pool = pool("q8pool", 3)
        hback = pool("hback", 4)
        hcs_pool = pool("hcs", 2 * (GSZ // 2) + 2)
        hq_pool = pool("hq", cc // 2 + 2)
        ypool = pool("ypool", 2)
        stats = pool("stats", 2)
        # ps_small: mm1 accumulators + PE transposes + row broadcasts
        ps_small = pool("ps_small", 4, "PSUM")
        ps_y = pool("ps_y", 4, "PSUM")

        ident_bf = const.tile([P, P], bf16, tag="idb")
        make_identity(nc, ident_bf)
        ident_f = const.tile([P, P], f32, tag="idf")
        make_identity(nc, ident_f)
        ones_f = const.tile([1, P], f32, tag="ones")
        nc.any.memset(ones_f[:], 1.0)

        def bcast_row(row_ap, width, out_tile):
            """Broadcast [1, width] row to all 128 partitions of out_tile."""
            off = 0
            for w in _split512(width):
                pb = ps_small.tile([P, w], f32, tag="mm1")
                nc.tensor.matmul(
                    pb[:], ones_f[:], row_ap[0:1, off : off + w],
                    start=True, stop=True,
                )
                nc.vector.tensor_copy(out_tile[:, off : off + w], pb[:])
                off += w

        # ---------------- stage 0: x load, act_quant, transpose ------------
        xqT = [
            xqt_pool.tile([P, text], bf16, tag="xqt", name=f"xqT{d}")
            for d in range(dc)
        ]
        alpha_cols = const.tile([P, tt], f32, tag="acols")
        nc.any.memset(alpha_cols[:], 0.0)

        def stage0_tile(t):
            p = min(P, text - t * P)
            xt = xload.tile([p, D], f32, tag="xt")
            nc.sync.dma_start(out=xt[:], in_=xe[t * P : t * P + p, :])
            m = xstat.tile([p, 1], f32, tag="m")
            nc.vector.tensor_reduce(
                m[:], xt[:], axis=AX.X, op=ALU.max, apply_absolute_value=True
            )
            # alpha column: clip(m, eps) * (inv_s1 / 127)
            nc.vector.tensor_scalar(
                alpha_cols[0:p, t : t + 1], m[:], EPS, alpha_c,
                op0=ALU.max, op1=ALU.mult,
            )
            mclip = xstat.tile([p, 1], f32, tag="mclip")
            nc.vector.tensor_scalar_max(mclip[:], m[:], EPS)
            rec = xstat.tile([p, 1], f32, tag="rec")
            nc.vector.reciprocal(rec[:], mclip[:])
            sx = xstat.tile([p, 1], f32, tag="sx")
            nc.vector.tensor_scalar_mul(sx[:], rec[:], 127.0)
            xq8 = xload.tile([p, D], i8, tag="xq8", bufs=2)
            nc.vector.tensor_scalar_mul(xq8[:], xt[:], sx[:])
            xqb = xload.tile([p, D], bf16, tag="xqb")
            nc.vector.tensor_copy(xqb[:], xq8[:])
            for d in range(dc):
                pt = ps_small.tile([P, p], bf16, tag="mm1")
                nc.tensor.transpose(
                    pt[:], xqb[:, d * P : (d + 1) * P], ident_bf[0:p, 0:p]
                )
                nc.scalar.activation(
                    xqT[d][:, t * P : t * P + p], pt[:], AF.Copy
                )

        for t in range(tt):
            stage0_tile(t)

        # alpha row -> broadcast to all 128 partitions
        apt = ps_small.tile([tt, P], f32, tag="mm1")
        nc.tensor.transpose(apt[:], alpha_cols[:], ident_f[:])
        arow9 = stats.tile([tt, P], f32, tag="arow9")
        nc.vector.tensor_copy(arow9[:], apt[:])
        arow = res.tile([1, tt * P], f32, tag="arow")
        nc.sync.dma_start(out=arow[:], in_=arow9[:])
        abc = res.tile([P, text], f32, tag="abc")
        bcast_row(arow, text, abc)

        # ---------------- per-half pipeline --------------------------------
        def sweep_a_pair(base, i):
            """mm1 + dequant + conv taps/adds for chunks (2i, 2i+1).

            Leaves the pre-silu conv sums (without bias) in a paired tile
            [P, 2, half]; bias is folded into the Silu activation later.
            Pairing halves the per-op overhead of the wide DVE ops.
            """
            ht2 = hpool.tile([P, 2, hext], f32, tag="ht")
            cwcs = []
            for j in range(2):
                c = 2 * i + j
                w1c = w1p.tile([P, dc, P], bf16, tag="w1c")
                nc.sync.dma_start(
                    out=w1c[:],
                    in_=w1s[c].rearrange("p (k m) -> p k m", k=dc),
                )
                for n in range(2):
                    noff = base + n * n1
                    pm = ps_small.tile([P, n1], f32, tag="mm1")
                    for d in range(dc):
                        nc.tensor.matmul(
                            pm[:],
                            w1c[:, d, :],
                            xqT[d][:, noff : noff + n1],
                            start=(d == 0),
                            stop=(d == dc - 1),
                        )
                    # dequant: h = H_int * alpha_t  (PSUM -> SBUF)
                    nc.vector.tensor_tensor(
                        ht2[:, j, n * n1 : (n + 1) * n1], pm[:],
                        abc[:, noff : noff + n1], op=ALU.mult,
                    )
                cwc = cwp.tile([P, 4], f32, tag="cwc")
                nc.sync.dma_start(out=cwc[:], in_=cw[c * P : (c + 1) * P, :])
                cwcs.append(cwc)
            ta2 = acttmp.tile([P, 2, half], f32, tag="atmp")
            tb2 = acttmp.tile([P, 2, half], f32, tag="atmp")
            tc2 = acttmp.tile([P, 2, half], f32, tag="atmp")
            for j in range(2):
                nc.scalar.activation(
                    ta2[:, j], ht2[:, j, 0:half], AF.Copy, scale=cwcs[j][:, 0:1]
                )
                nc.scalar.activation(
                    tb2[:, j], ht2[:, j, 2 : 2 + half], AF.Copy,
                    scale=cwcs[j][:, 2:3],
                )
                nc.scalar.activation(
                    tc2[:, j], ht2[:, j, 1 : 1 + half], AF.Copy,
                    scale=cwcs[j][:, 1:2],
                )
            hcs2 = hcs_pool.tile([P, 2, half], f32, tag="hcs")
            nc.vector.tensor_add(ta2[:], ta2[:], tb2[:])
            nc.vector.tensor_add(hcs2[:], ta2[:], tc2[:])
            return hcs2, cwcs

        def silu_abs_pairs(hf, grp, macc2):
            """Batched Silu (in-place, +bias), paired Abs/max, spill to DRAM."""
            for _, hcs2, cwcs in grp:
                for j in range(2):
                    if sim_silu:
                        sg = acttmp.tile([P, half], f32, tag="atmp")
                        nc.scalar.activation(
                            sg[:], hcs2[:, j], AF.Sigmoid, bias=cwcs[j][:, 3:4]
                        )
                        nc.vector.tensor_scalar(
                            hcs2[:, j], hcs2[:, j], cwcs[j][:, 3:4], None,
                            op0=ALU.add,
                        )
                        nc.vector.tensor_tensor(
                            hcs2[:, j], hcs2[:, j], sg[:], op=ALU.mult
                        )
                    else:
                        nc.scalar.activation(
                            hcs2[:, j], hcs2[:, j], AF.Silu, bias=cwcs[j][:, 3:4]
                        )
            for i, hcs2, _ in grp:
                habs2 = acttmp.tile([P, 2, half], f32, tag="atmp")
                nc.scalar.activation(habs2[:], hcs2[:], AF.Abs)
                nc.vector.tensor_tensor(macc2[:], macc2[:], habs2[:], op=ALU.max)
                nc.sync.dma_start(
                    out=hspill[hf, 2 * i * P : (2 * i + 2) * P, :].rearrange(
                        "(j p) t -> p j t", p=P
                    ),
                    in_=hcs2[:],
                )

        def token_scales(macc2):
            """Per-token |h| max -> (beta_cols [P, mt], shbc [P, half])."""
            macc = stats.tile([P, half], f32, tag="maccf", bufs=1)
            nc.vector.tensor_tensor(
                macc[:], macc2[:, 0], macc2[:, 1], op=ALU.max
            )
            mh_cols = stats.tile([P, mt], f32, tag="mhcols")
            for n4 in range(mt):
                pt = ps_small.tile([P, P], f32, tag="mm1")
                nc.tensor.transpose(
                    pt[:], macc[:, n4 * P : (n4 + 1) * P], ident_f[:]
                )
                nc.vector.tensor_reduce(
                    mh_cols[:, n4 : n4 + 1], pt[:], axis=AX.X, op=ALU.max
                )
            nc.vector.tensor_scalar_max(mh_cols[:], mh_cols[:], EPS)
            beta_cols = stats.tile([P, mt], f32, tag="bcols")
            nc.vector.tensor_scalar_mul(beta_cols[:], mh_cols[:], beta_c)
            rec4 = stats.tile([P, mt], f32, tag="rec4")
            nc.vector.reciprocal(rec4[:], mh_cols[:])
            shcols = stats.tile([P, mt], f32, tag="shcols")
            nc.vector.tensor_scalar_mul(shcols[:], rec4[:], 127.0)
            spt = ps_small.tile([mt, P], f32, tag="mm1")
            nc.tensor.transpose(spt[:], shcols[:], ident_f[:])
            sh4 = stats.tile([mt, P], f32, tag="sh4")
            nc.vector.tensor_copy(sh4[:], spt[:])
            shrow = stats.tile([1, half], f32, tag="shrow")
            nc.sync.dma_start(out=shrow[:], in_=sh4[:])
            shbc = stats.tile([P, half], f32, tag="shbc")
            bcast_row(shrow, half, shbc)
            return beta_cols, shbc

        def quant_half(hf, shbc, hq_tiles):
            for i in range(cc // 2):
                hb2 = hback.tile([P, 2, half], f32, tag="hb")
                nc.sync.dma_start(
                    out=hb2[:],
                    in_=hspill[hf, 2 * i * P : (2 * i + 2) * P, :].rearrange(
                        "(j p) t -> p j t", p=P
                    ),
                )
                hq82 = q8pool.tile([P, 2, half], i8, tag="hq8")
                for j in range(2):
                    nc.vector.tensor_tensor(
                        hq82[:, j], hb2[:, j], shbc[:], op=ALU.mult
                    )
                hqb2 = hq_pool.tile([P, 2, half], bf16, tag="hqb")
                nc.vector.tensor_copy(hqb2[:], hq82[:])
                hq_tiles.append(hqb2)

        def mm2_pass(base, n, beta_cols, hq_tiles):
            psy = [
                ps_y.tile([P, 512], f32, tag="yacc", name=f"psy{m_}")
                for m_ in range(mt)
            ]
            for c in range(cc):
                w2c = w2p.tile([P, 512], bf16, tag="w2c")
                nc.sync.dma_start(
                    out=w2c[:],
                    in_=w2t[c * P : (c + 1) * P, n * 512 : (n + 1) * 512],
                )
                for m_ in range(mt):
                    nc.tensor.matmul(
                        psy[m_][:],
                        hq_tiles[c // 2][:, c % 2, m_ * P : (m_ + 1) * P],
                        w2c[:],
                        start=(c == 0),
                        stop=(c == cc - 1),
                    )
            for m_ in range(mt):
                ysb = ypool.tile([P, 512], f32, tag="ysb")
                # evict on DVE so ACT backlog never blocks PSUM reuse
                nc.vector.tensor_scalar_mul(
                    ysb[:], psy[m_][:], beta_cols[:, m_ : m_ + 1]
                )
                nc.sync.dma_start(
                    out=y_ext[
                        base + m_ * P : base + (m_ + 1) * P,
                        n * 512 : (n + 1) * 512,
                    ],
                    in_=ysb[:],
                )
            return

        npairs = cc // 2
        gp = max(1, GSZ // 2)  # pairs per ACT phase group
        groups = []
        g0 = 0
        while g0 < npairs:
            rem = npairs - g0
            sz = gp if rem > 2 * gp else max(1, rem // 2)
            groups.append((g0, min(g0 + sz, npairs)))
            g0 += sz

        state = []
        for hf in range(2):
            macc2 = const.tile([P, 2, half], f32, tag="macc2", bufs=2)
            nc.any.memset(macc2[:], 0.0)
            state.append({"pairs": [], "macc2": macc2, "prev": None})

        def emit_groups(hf, lo, hi):
            """Emit sweep-A groups [lo, hi) of half hf, silu/abs pipelined
            one group behind."""
            st = state[hf]
            base = hf * half
            for gi in range(lo, hi):
                g0, g1 = groups[gi]
                for i in range(g0, g1):
                    hcs2, cwcs = sweep_a_pair(base, i)
                    st["pairs"].append((i, hcs2, cwcs))
                if st["prev"] is not None:
                    p0, p1 = st["prev"]
                    silu_abs_pairs(hf, st["pairs"][p0:p1], st["macc2"])
                st["prev"] = (g0, g1)

        def finish_groups(hf):
            st = state[hf]
            p0, p1 = st["prev"]
            silu_abs_pairs(hf, st["pairs"][p0:p1], st["macc2"])

        ng = len(groups)
        # Interleaved emission: each half's PE-heavy mm2 passes are woven
        # between the other half's DVE/ACT-heavy conv-sweep groups, so the
        # in-order PE stream always has matmul work while DVE/ACT stream
        # conv work, and vice versa.
        emit_groups(0, 0, ng)
        finish_groups(0)
        emit_groups(1, 0, 1)
        beta0, shbc0 = token_scales(state[0]["macc2"])
        hq0 = []
        quant_half(0, shbc0, hq0)
        mm2_pass(0, 0, beta0, hq0)
        emit_groups(1, 1, max(1, ng - 1))
        mm2_pass(0, 1, beta0, hq0)
        emit_groups(1, max(1, ng - 1), ng)
        finish_groups(1)
        beta1, shbc1 = token_scales(state[1]["macc2"])
        hq1 = []
        quant_half(1, shbc1, hq1)
        mm2_pass(half, 0, beta1, hq1)
        mm2_pass(half, 1, beta1, hq1)
    return nc


def _host_prep(x, w1, conv_w, conv_b, w2, t_own):
    """Quantize weights and build per-core halo-padded x slabs."""
    bf16 = ml_dtypes.bfloat16
    cc, dc = H // P, D // P
    s1inv = np.maximum(np.mean(np.abs(w1)), np.float32(EPS)).astype(np.float32)
    scale1 = np.float32(1.0) / s1inv
    w1q = np.clip(np.rint(w1 * scale1), -1, 1).astype(np.float32)
    s2inv = np.maximum(np.mean(np.abs(w2)), np.float32(EPS)).astype(np.float32)
    scale2 = np.float32(1.0) / s2inv
    w2q = np.clip(np.rint(w2 * scale2), -1, 1).astype(np.float32)

    # w1s[c, p, k*128+m] = w1q[c*128+m, k*128+p] -> per-chunk contiguous lhsT
    w1s = np.ascontiguousarray(
        w1q.reshape(cc, P, dc, P).transpose(0, 3, 2, 1).reshape(cc, P, D)
    ).astype(bf16)
    w2t = np.ascontiguousarray(w2q.T).astype(bf16)          # [H, D]
    cw = np.stack(
        [conv_w[:, 0, 0], conv_w[:, 0, 1], conv_w[:, 0, 2], conv_b], axis=1
    ).astype(np.float32)                                     # [H, 4]

    n_cores = x.shape[0] * x.shape[1] // t_own
    xf = x.reshape(-1, x.shape[-1])
    slabs = []
    for c in range(n_cores):
        xe = np.zeros((t_own + 2, xf.shape[1]), np.float32)
        lo = c * t_own
        xe[1 : 1 + t_own] = xf[lo : lo + t_own]
        if lo % S != 0:
            xe[0] = xf[lo - 1]
        if (lo + t_own) % S != 0 and lo + t_own < xf.shape[0]:
            xe[1 + t_own] = xf[lo + t_own]
        slabs.append(xe)

    alpha_c = float(s1inv) / 127.0
    beta_c = float(s2inv) / 127.0
    return w1s, w2t, cw, slabs, alpha_c, beta_c


def _run(x, w1, conv_w, conv_b, w2, trace=False, **spmd_kwargs):
    import sys
    if "/opt/trn_rl_repo" not in sys.path:
        sys.path.append("/opt/trn_rl_repo")
    _install_tile_patch()
    from concourse.bass_utils import run_bass_kernel_spmd

    t_own = x.shape[0] * x.shape[1] // N_CORES
    w1s, w2t, cw, slabs, alpha_c, beta_c = _host_prep(
        x, w1, conv_w, conv_b, w2, t_own
    )
    nc = build_nc(t_own, alpha_c, beta_c)
    in_maps = [
        {"xe": slabs[c], "w1s": w1s, "w2t": w2t, "cw": cw}
        for c in range(N_CORES)
    ]
    out = run_bass_kernel_spmd(
        nc, in_maps, list(range(N_CORES)), trace=trace, **spmd_kwargs
    )
    y = np.concatenate([out.results[c]["y"] for c in range(N_CORES)], axis=0)
    y = np.ascontiguousarray(y.reshape(x.shape[0], x.shape[1], -1))
    return y, out


def kernel(x, w1, conv_w, conv_b, w2):
    return _run(x, w1, conv_w, conv_b, w2)[0]

